# revision 17
# baseline (speedup 1.0000x reference)
"""Trainium2 Bass kernel: Lap-regularizer gradient step (graph Laplacian).

out = z - COEFF * grad,  grad = (2/N) * norm ⊙ (deg·z_reg - A_sym·z_reg),
z_reg = norm ⊙ z, A_sym = symmetrized adjacency from edge_index.

Strategy (8 NeuronCores, SPMD, no collectives):
  - nodes sharded 12500/core; the 3.2M symmetrized directed edges are
    bucketed by dst core, grouped by (128-dst group, aligned 32-dst
    quarter), sorted, and packed into 128-edge chunks (partition-major).
  - per-edge streams from host (indexing/casts only, no arithmetic):
    z[src] rows (fp8, 48B), dst_rel (int16, 2B), norm[src] (bf16, 2B).
  - device builds the norm-scaled one-hot selectors with two DVE ops per
    stream tile (is_equal against a materialized iota, then mult), then
    segment-sums via bf16-sel x fp8-z matmuls accumulating in PSUM.
  - PSUM: one 512-f32 bank accumulates 10 dst groups (has_written
    overwrite-then-accumulate semantics); one DVE drain per bank with
    the C2*norm_dst scale fused; quarters issue round-robin so
    consecutive matmuls target different PE column groups.
"""

import os

import numpy as np
import ml_dtypes

import concourse.bass as bass
import concourse.mybir as mybir
import concourse.tile as tile
from concourse import bacc
from concourse._compat import get_trn_type
from concourse.bass_utils import run_bass_kernel_spmd

# Problem constants (hardcoded; kernel.py must be self-contained).
N = 100000
D = 48
COEFF = 0.1
C2 = COEFF * 2.0 / N

M = 8                      # cores
NPC = N // M               # nodes per core
P = 128
NGRP = (NPC + P - 1) // P  # 98 dst groups per core
W = 32                     # one-hot window = aligned 32-dst quarter
GS = 256                   # stream slots (of 128 edges) per DMA tile
GPB = 10                   # dst groups per PSUM bank (10*48=480 <= 512)

F32 = mybir.dt.float32
BF16 = mybir.dt.bfloat16
FP8 = mybir.dt.float8e4
I16 = mybir.dt.int16

LAST_RESULTS = None


def _slot_schedule(slots_gq):
    """Round-robin-quarter slot order, shared by host packing and the
    device graph. Returns (slot_list, slot_of[g][q][j])."""
    slot_list = []
    slot_of = {}
    for g in range(NGRP):
        mx = int(slots_gq[g].max())
        for j in range(mx):
            for q in range(4):
                if j < int(slots_gq[g, q]):
                    slot_of[(g, q, j)] = len(slot_list)
                    slot_list.append((g, q, j))
    return slot_list, slot_of


def _preprocess(z, edge_index, norm_factor):
    """Host-side sharding/packing. Returns per-core input maps + metadata."""
    ei = np.asarray(edge_index).astype(np.int64)
    row, col = ei[0], ei[1]
    src_all = np.concatenate([row, col])
    dst_all = np.concatenate([col, row])
    ne = src_all.shape[0]

    deg_all = np.bincount(dst_all, minlength=N)
    # pack each core's nodes into 392 windows of <=32 nodes, balancing the
    # per-window edge counts (LPT) so chunk counts are uniform across
    # (core, group, quarter) buckets -> minimal stream padding
    import heapq

    NW = NGRP * 4
    NOVER = 12                # big windows absorbing the highest-degree nodes
    ECAP = 8 * 128            # edge cap keeping regular windows at 8 chunks
    perm = np.empty(N, np.int64)   # node -> local slot within its core
    for c in range(M):
        ids = np.arange(c * NPC, (c + 1) * NPC)
        order_d = ids[np.argsort(-deg_all[ids], kind="stable")]
        counts = np.zeros(NW, np.int64)
        # top-degree nodes fill the overflow windows outright
        for i, v in enumerate(order_d[: NOVER * 32]):
            w = i // 32
            perm[v] = (w // 4) * 128 + (w % 4) * 32 + counts[w]
            counts[w] += 1
        # remaining nodes: LPT over capped windows (reject-on-overflow)
        heap = [(0, w) for w in range(NOVER, NW)]
        heapq.heapify(heap)
        for v in order_d[NOVER * 32 :]:
            dv = int(deg_all[v])
            stash = []
            while True:
                if not heap:
                    # infeasible tail: fall back to least-loaded stashed
                    heap = stash
                    heapq.heapify(heap)
                    stash = []
                    ssum, w = heapq.heappop(heap)
                    break
                ssum, w = heapq.heappop(heap)
                if counts[w] >= 32:
                    continue
                if ssum + dv > ECAP:
                    stash.append((ssum, w))
                    continue
                break
            for e in stash:
                heapq.heappush(heap, e)
            perm[v] = (w // 4) * 128 + (w % 4) * 32 + counts[w]
            counts[w] += 1
            if counts[w] < 32:
                heapq.heappush(heap, (ssum + dv, w))

    core = dst_all // NPC
    dloc = perm[dst_all]
    grp = dloc >> 7
    pdst = dloc & 127
    qrt = pdst >> 5

    # bucket = (core, group, quarter); chunks never straddle quarters so the
    # matmul PSUM base partition stays 32-aligned.
    NB = NGRP * 4
    key = (core * NGRP + grp) * 4 + qrt
    cnt = np.bincount(key, minlength=M * NB)
    bstart = np.zeros(M * NB + 1, np.int64)
    np.cumsum(cnt, out=bstart[1:])

    order = np.argsort(key, kind="stable")
    key_s = key[order]
    pdst_s = pdst[order]
    src_s = src_all[order]
    core_s = core[order]
    j_in = np.arange(ne, dtype=np.int64) - bstart[key_s]
    chunk_s = j_in >> 7
    p_s = (j_in & 127).astype(np.int64)

    # unified slot layout across cores: slots per (group, quarter), with a
    # >=1 floor so every PSUM element is written at least once (no stale
    # PSUM reads at drains)
    K = ((cnt + 127) // 128).reshape(M, NB)
    slots_gq = np.maximum(K.max(axis=0), 1).reshape(NGRP, 4)

    slot_list, slot_of = _slot_schedule(slots_gq)
    SLOTS = len(slot_list)
    SL = np.zeros((NGRP, 4, int(slots_gq.max())), np.int64)
    for (g, q, j), s in slot_of.items():
        SL[g, q, j] = s

    grp_s = key_s // 4 % NGRP
    qrt_s = key_s % 4
    slot_s = SL[grp_s, qrt_s, chunk_s]
    dl_rel = (pdst_s & 31).astype(np.int64)

    zf = np.asarray(z, np.float32)
    nf = np.asarray(norm_factor, np.float32).reshape(-1)
    zb = zf.astype(ml_dtypes.float8_e4m3)
    n8 = nf.astype(ml_dtypes.float8_e4m3)

    # per-edge source streams (host gather = indexing/casting only):
    # z rows (fp8) + norm[src]-scaled one-hot selector chunks (fp8)
    zs_arr = np.zeros((M, P, SLOTS, D), ml_dtypes.float8_e4m3)
    sp_arr = np.zeros((M, P, SLOTS * W), ml_dtypes.float8_e4m3)
    zs_arr[core_s, p_s, slot_s] = zb[src_s]
    sp_arr[core_s, p_s, slot_s * W + dl_rel] = n8[src_s]

    deg = deg_all.astype(np.float32)
    node_core = np.arange(N) // NPC

    def core_layout(x, width, dt):
        xp = np.zeros((M, NGRP * P, width), dt)
        xp[node_core, perm] = x.reshape(N, width).astype(dt)
        return (
            xp.reshape(M, NGRP, P, width)
            .transpose(0, 2, 1, 3)
            .reshape(M, P, NGRP * width)
        )

    zl_arr = core_layout(zf, D, ml_dtypes.bfloat16)
    nl_arr = core_layout(nf.reshape(N, 1), 1, np.float32)
    dg_arr = core_layout(deg.reshape(N, 1), 1, np.float32)

    in_maps = []
    for c in range(M):
        in_maps.append(
            {
                "zs": np.ascontiguousarray(zs_arr[c]).reshape(P, SLOTS * D),
                "sp": np.ascontiguousarray(sp_arr[c]),
                "zl": np.ascontiguousarray(zl_arr[c]),
                "nl": np.ascontiguousarray(nl_arr[c]),
                "dg": np.ascontiguousarray(dg_arr[c]),
            }
        )

    meta = {"perm": perm, "SLOTS": SLOTS, "slots_gq": slots_gq}
    return in_maps, meta


def build_graph(meta):
    SLOTS = meta["SLOTS"]
    slots_gq = meta["slots_gq"]
    slot_list, _ = _slot_schedule(slots_gq)
    assert len(slot_list) == SLOTS

    # group -> psum superbank id; first/last slot ids per (bank, L/H half)
    bank_of = [g // GPB for g in range(NGRP)]
    first_slot = {}
    last_slot = {}
    last_any = {}
    for s, (g, q, j) in enumerate(slot_list):
        b = bank_of[g]
        h = 0 if q < 2 else 1
        if (b, h) not in first_slot:
            first_slot[(b, h)] = s
        last_slot[(b, h)] = s
        last_any[b] = s

    nc = bacc.Bacc(
        get_trn_type() or "TRN2",
        target_bir_lowering=False,
        debug=False,
        num_devices=M,
    )

    zs_d = nc.dram_tensor("zs", [P, SLOTS * D], FP8, kind="ExternalInput")
    sp_d = nc.dram_tensor("sp", [P, SLOTS * W], FP8, kind="ExternalInput")
    zl_d = nc.dram_tensor("zl", [P, NGRP * D], BF16, kind="ExternalInput")
    nl_d = nc.dram_tensor("nl", [P, NGRP], F32, kind="ExternalInput")
    dg_d = nc.dram_tensor("dg", [P, NGRP], F32, kind="ExternalInput")
    out_d = nc.dram_tensor("out", [P, NGRP * D], BF16, kind="ExternalOutput")

    with tile.TileContext(nc) as tc:
        with tc.tile_pool(name="persist", bufs=1) as pp, tc.tile_pool(
            name="zstream", bufs=3
        ) as zp, tc.tile_pool(
            name="sstream", bufs=3
        ) as spp, tc.tile_pool(
            name="psum", bufs=4, space="PSUM"
        ) as ppool:
            zl_sb = pp.tile([P, NGRP * D], BF16)
            nc.scalar.dma_start(zl_sb[:], zl_d.ap())
            nl_sb = pp.tile([P, NGRP], F32)
            nc.scalar.dma_start(nl_sb[:], nl_d.ap())
            dg_sb = pp.tile([P, NGRP], F32)
            nc.scalar.dma_start(dg_sb[:], dg_d.ap())
            out_sb = pp.tile([P, NGRP * D], BF16)
            nbr_sb = pp.tile([P, NGRP * D], BF16)

            # m = 1 - C2*deg*norm^2 ; b = C2*norm
            m_sb = pp.tile([P, NGRP], F32)
            b_sb = pp.tile([P, NGRP], F32)
            nc.vector.tensor_tensor(
                out=m_sb[:], in0=nl_sb[:], in1=nl_sb[:], op=mybir.AluOpType.mult
            )
            nc.vector.tensor_tensor(
                out=m_sb[:], in0=m_sb[:], in1=dg_sb[:], op=mybir.AluOpType.mult
            )
            nc.vector.tensor_scalar(
                out=m_sb[:],
                in0=m_sb[:],
                scalar1=-C2,
                scalar2=1.0,
                op0=mybir.AluOpType.mult,
                op1=mybir.AluOpType.add,
            )
            nc.vector.tensor_scalar(
                out=b_sb[:],
                in0=nl_sb[:],
                scalar1=C2,
                scalar2=None,
                op0=mybir.AluOpType.mult,
            )

            zst = None
            selt = None
            pt = None
            for s, (g, q, j) in enumerate(slot_list):
                gi, k = divmod(s, GS)
                if k == 0:
                    gs = min(GS, SLOTS - s)
                    zst = zp.tile([P, GS, D], FP8, tag="zst")
                    nc.sync.dma_start(
                        zst[:, 0:gs, :].rearrange("p a b -> p (a b)"),
                        zs_d.ap()[:, s * D : (s + gs) * D],
                    )
                    selt = spp.tile([P, GS, W], FP8, tag="selt")
                    nc.scalar.dma_start(
                        selt[:, 0:gs, :].rearrange("p a b -> p (a b)"),
                        sp_d.ap()[:, s * W : (s + gs) * W],
                    )
                b = bank_of[g]
                h = 0 if q < 2 else 1
                if s == first_slot[(b, 0)]:
                    ptL = ppool.tile([64, 512], F32, tag="ptL")
                    ptH = ppool.tile([64, 512], F32, tag="ptH")
                    # zero the banks; all matmuls then accumulate with
                    # start=False (correct whether stale has_written bits
                    # are set or not, for any matmul issue order)
                    ngb = (min(NGRP, b * GPB + GPB) - b * GPB) * D
                    nc.vector.memset(ptL[:, 0:ngb], 0.0)
                    nc.vector.memset(ptH[:, 0:ngb], 0.0)
                g0 = b * GPB
                cols = slice((g - g0) * D, (g - g0 + 1) * D)
                pt = ptL if h == 0 else ptH
                off = (q % 2) * W
                nc.tensor.matmul(
                    pt[off : off + W, cols],
                    selt[:, k, :],
                    zst[:, k, :],
                    start=False,
                    stop=(s == last_slot[(b, h)]),
                    skip_group_check=True,
                )
                if s == last_any[b]:
                    # drain bank: nbr = psum * b  (C2*norm_dst), then
                    # out = zl*m + nbr for these groups, stream out
                    g1 = min(NGRP, g0 + GPB)
                    ng = g1 - g0
                    ccols = slice(g0 * D, g1 * D)
                    for half, ptx in ((0, ptL), (1, ptH)):
                        prow = slice(half * 64, half * 64 + 64)
                        p3 = ptx[:, 0 : ng * D].rearrange(
                            "p (a b) -> p a b", b=D
                        )
                        n3 = nbr_sb[prow, ccols].rearrange(
                            "p (a b) -> p a b", b=D
                        )
                        nc.vector.tensor_tensor(
                            out=n3,
                            in0=p3,
                            in1=b_sb[prow, g0:g1].to_broadcast([64, ng, D]),
                            op=mybir.AluOpType.mult,
                        )
                    zl3 = zl_sb[:, ccols].rearrange("p (a b) -> p a b", b=D)
                    o3 = out_sb[:, ccols].rearrange("p (a b) -> p a b", b=D)
                    nc.vector.tensor_tensor(
                        out=o3,
                        in0=zl3,
                        in1=m_sb[:, g0:g1].to_broadcast([P, ng, D]),
                        op=mybir.AluOpType.mult,
                    )
                    nc.vector.tensor_tensor(
                        out=out_sb[:, ccols],
                        in0=out_sb[:, ccols],
                        in1=nbr_sb[:, ccols],
                        op=mybir.AluOpType.add,
                    )
                    nc.scalar.dma_start(out_d.ap()[:, ccols], out_sb[:, ccols])

    return nc


def kernel(**inputs):
    global LAST_RESULTS
    z = np.asarray(inputs["z"], np.float32)
    edge_index = inputs["edge_index"]
    norm_factor = np.asarray(inputs["norm_factor"], np.float32)

    in_maps, meta = _preprocess(z, edge_index, norm_factor)

    nc = build_graph(meta)
    nc.compile()

    trace = os.environ.get("KERNEL_TRACE", "0") == "1"
    res = run_bass_kernel_spmd(
        nc, in_maps, core_ids=list(range(M)), trace=trace
    )
    LAST_RESULTS = res

    perm = meta["perm"]
    result = np.empty((N, D), np.float32)
    for c in range(M):
        o = np.asarray(res.results[c]["out"], np.float32)
        o = o.reshape(P, NGRP, D).transpose(1, 0, 2).reshape(NGRP * P, D)
        ids = np.arange(c * NPC, (c + 1) * NPC)
        result[ids] = o[perm[ids]]
    return result


# revision 23
# speedup vs baseline: 1.0926x; 1.0926x over previous
"""Trainium2 Bass kernel: Lap-regularizer gradient step (graph Laplacian).

out = z - COEFF * grad,  grad = (2/N) * norm ⊙ (deg·z_reg - A_sym·z_reg),
z_reg = norm ⊙ z, A_sym = symmetrized adjacency from edge_index.

Strategy (8 NeuronCores, SPMD, no collectives):
  - nodes sharded 12500/core; the 3.2M symmetrized directed edges are
    bucketed by dst core, grouped by (128-dst group, aligned 32-dst
    quarter), sorted, and packed into 128-edge chunks (partition-major).
  - per-edge streams from host (indexing/casts only, no arithmetic):
    z[src] rows (fp8, 48B), dst_rel (int16, 2B), norm[src] (bf16, 2B).
  - device builds the norm-scaled one-hot selectors with two DVE ops per
    stream tile (is_equal against a materialized iota, then mult), then
    segment-sums via bf16-sel x fp8-z matmuls accumulating in PSUM.
  - PSUM: one 512-f32 bank accumulates 10 dst groups (has_written
    overwrite-then-accumulate semantics); one DVE drain per bank with
    the C2*norm_dst scale fused; quarters issue round-robin so
    consecutive matmuls target different PE column groups.
"""

import os

import numpy as np
import ml_dtypes

import concourse.bass as bass
import concourse.mybir as mybir
import concourse.tile as tile
from concourse import bacc
from concourse._compat import get_trn_type
from concourse.bass_utils import run_bass_kernel_spmd

# Problem constants (hardcoded; kernel.py must be self-contained).
N = 100000
D = 48
COEFF = 0.1
C2 = COEFF * 2.0 / N

M = 8                      # cores
NPC = N // M               # nodes per core
P = 128
NGRP = (NPC + P - 1) // P  # 98 dst groups per core
W = 32                     # one-hot window = aligned 32-dst quarter
GS = 256                   # stream slots (of 128 edges) per DMA tile
GPB = 10                   # dst groups per PSUM bank (10*48=480 <= 512)

F32 = mybir.dt.float32
BF16 = mybir.dt.bfloat16
FP8 = mybir.dt.float8e4
I16 = mybir.dt.int16

LAST_RESULTS = None


def _slot_schedule(slots_gq):
    """Quarter-sequential slot order, shared by host packing and the
    device graph. Returns (slot_list, slot_of[g][q][j])."""
    slot_list = []
    slot_of = {}
    for g in range(NGRP):
        for q in range(4):
            for j in range(int(slots_gq[g, q])):
                slot_of[(g, q, j)] = len(slot_list)
                slot_list.append((g, q, j))
    return slot_list, slot_of


def _preprocess(z, edge_index, norm_factor):
    """Host-side sharding/packing. Returns per-core input maps + metadata."""
    ei = np.asarray(edge_index).astype(np.int64)
    row, col = ei[0], ei[1]
    src_all = np.concatenate([row, col])
    dst_all = np.concatenate([col, row])
    ne = src_all.shape[0]

    deg_all = np.bincount(dst_all, minlength=N)
    # pack each core's nodes into 392 windows of <=32 nodes, balancing the
    # per-window edge counts (LPT) so chunk counts are uniform across
    # (core, group, quarter) buckets -> minimal stream padding
    import heapq

    NW = NGRP * 4
    NOVER = 12                # big windows absorbing the highest-degree nodes
    ECAP = 8 * 128            # edge cap keeping regular windows at 8 chunks
    perm = np.empty(N, np.int64)   # node -> local slot within its core
    for c in range(M):
        ids = np.arange(c * NPC, (c + 1) * NPC)
        order_d = ids[np.argsort(-deg_all[ids], kind="stable")]
        counts = np.zeros(NW, np.int64)
        # top-degree nodes fill the overflow windows outright
        for i, v in enumerate(order_d[: NOVER * 32]):
            w = i // 32
            perm[v] = (w // 4) * 128 + (w % 4) * 32 + counts[w]
            counts[w] += 1
        # remaining nodes: LPT over capped windows (reject-on-overflow)
        heap = [(0, w) for w in range(NOVER, NW)]
        heapq.heapify(heap)
        for v in order_d[NOVER * 32 :]:
            dv = int(deg_all[v])
            stash = []
            while True:
                if not heap:
                    # infeasible tail: fall back to least-loaded stashed
                    heap = stash
                    heapq.heapify(heap)
                    stash = []
                    ssum, w = heapq.heappop(heap)
                    break
                ssum, w = heapq.heappop(heap)
                if counts[w] >= 32:
                    continue
                if ssum + dv > ECAP:
                    stash.append((ssum, w))
                    continue
                break
            for e in stash:
                heapq.heappush(heap, e)
            perm[v] = (w // 4) * 128 + (w % 4) * 32 + counts[w]
            counts[w] += 1
            if counts[w] < 32:
                heapq.heappush(heap, (ssum + dv, w))

    core = dst_all // NPC
    dloc = perm[dst_all]
    grp = dloc >> 7
    pdst = dloc & 127
    qrt = pdst >> 5

    # bucket = (core, group, quarter); chunks never straddle quarters so the
    # matmul PSUM base partition stays 32-aligned.
    NB = NGRP * 4
    key = (core * NGRP + grp) * 4 + qrt
    cnt = np.bincount(key, minlength=M * NB)
    bstart = np.zeros(M * NB + 1, np.int64)
    np.cumsum(cnt, out=bstart[1:])

    order = np.argsort(key, kind="stable")
    key_s = key[order]
    pdst_s = pdst[order]
    src_s = src_all[order]
    core_s = core[order]
    j_in = np.arange(ne, dtype=np.int64) - bstart[key_s]
    chunk_s = j_in >> 7
    p_s = (j_in & 127).astype(np.int64)

    # unified slot layout across cores: slots per (group, quarter), with a
    # >=1 floor so every PSUM element is written at least once (no stale
    # PSUM reads at drains)
    K = ((cnt + 127) // 128).reshape(M, NB)
    slots_gq = np.maximum(K.max(axis=0), 1).reshape(NGRP, 4)

    slot_list, slot_of = _slot_schedule(slots_gq)
    SLOTS = len(slot_list)
    SL = np.zeros((NGRP, 4, int(slots_gq.max())), np.int64)
    for (g, q, j), s in slot_of.items():
        SL[g, q, j] = s

    grp_s = key_s // 4 % NGRP
    qrt_s = key_s % 4
    slot_s = SL[grp_s, qrt_s, chunk_s]
    dl_rel = (pdst_s & 31).astype(np.int64)

    zf = np.asarray(z, np.float32)
    nf = np.asarray(norm_factor, np.float32).reshape(-1)
    zb = zf.astype(ml_dtypes.float8_e4m3)
    n8 = nf.astype(ml_dtypes.float8_e4m3)

    # per-edge source streams (host gather = indexing/casting only):
    # z rows (fp8) + norm[src]-scaled one-hot selector chunks (fp8)
    zs_arr = np.zeros((M, P, SLOTS, D), ml_dtypes.float8_e4m3)
    sp_arr = np.zeros((M, P, SLOTS * W), ml_dtypes.float8_e4m3)
    zs_arr[core_s, p_s, slot_s] = zb[src_s]
    sp_arr[core_s, p_s, slot_s * W + dl_rel] = n8[src_s]

    deg = deg_all.astype(np.float32)
    node_core = np.arange(N) // NPC

    def core_layout(x, width, dt):
        xp = np.zeros((M, NGRP * P, width), dt)
        xp[node_core, perm] = x.reshape(N, width).astype(dt)
        return (
            xp.reshape(M, NGRP, P, width)
            .transpose(0, 2, 1, 3)
            .reshape(M, P, NGRP * width)
        )

    zl_arr = core_layout(zf, D, np.float32)
    nl_arr = core_layout(nf.reshape(N, 1), 1, np.float32)
    dg_arr = core_layout(deg.reshape(N, 1), 1, np.float32)

    in_maps = []
    for c in range(M):
        in_maps.append(
            {
                "zs": np.ascontiguousarray(zs_arr[c]).reshape(P, SLOTS * D),
                "sp": np.ascontiguousarray(sp_arr[c]),
                "zl": np.ascontiguousarray(zl_arr[c]),
                "nl": np.ascontiguousarray(nl_arr[c]),
                "dg": np.ascontiguousarray(dg_arr[c]),
            }
        )

    meta = {"perm": perm, "SLOTS": SLOTS, "slots_gq": slots_gq}
    return in_maps, meta


def build_graph(meta):
    SLOTS = meta["SLOTS"]
    slots_gq = meta["slots_gq"]
    slot_list, _ = _slot_schedule(slots_gq)
    assert len(slot_list) == SLOTS

    # group -> psum superbank id; first/last slot ids per (bank, L/H half)
    bank_of = [g // GPB for g in range(NGRP)]
    first_slot = {}
    last_slot = {}
    last_any = {}
    for s, (g, q, j) in enumerate(slot_list):
        b = bank_of[g]
        h = 0 if q < 2 else 1
        if (b, h) not in first_slot:
            first_slot[(b, h)] = s
        last_slot[(b, h)] = s
        last_any[b] = s

    nc = bacc.Bacc(
        get_trn_type() or "TRN2",
        target_bir_lowering=False,
        debug=False,
        num_devices=M,
    )

    zs_d = nc.dram_tensor("zs", [P, SLOTS * D], FP8, kind="ExternalInput")
    sp_d = nc.dram_tensor("sp", [P, SLOTS * W], FP8, kind="ExternalInput")
    zl_d = nc.dram_tensor("zl", [P, NGRP * D], F32, kind="ExternalInput")
    nl_d = nc.dram_tensor("nl", [P, NGRP], F32, kind="ExternalInput")
    dg_d = nc.dram_tensor("dg", [P, NGRP], F32, kind="ExternalInput")
    out_d = nc.dram_tensor("out", [P, NGRP * D], F32, kind="ExternalOutput")

    with tile.TileContext(nc) as tc:
        with tc.tile_pool(name="persist", bufs=1) as pp, tc.tile_pool(
            name="zstream", bufs=3
        ) as zp, tc.tile_pool(
            name="sstream", bufs=3
        ) as spp, tc.tile_pool(
            name="psum", bufs=4, space="PSUM"
        ) as ppool:
            zl_sb = pp.tile([P, NGRP * D], F32)
            nc.scalar.dma_start(zl_sb[:], zl_d.ap())
            nl_sb = pp.tile([P, NGRP], F32)
            nc.scalar.dma_start(nl_sb[:], nl_d.ap())
            dg_sb = pp.tile([P, NGRP], F32)
            nc.scalar.dma_start(dg_sb[:], dg_d.ap())
            out_sb = pp.tile([P, NGRP * D], F32)
            nbr_sb = pp.tile([P, NGRP * D], BF16)

            # m = 1 - C2*deg*norm^2 ; b = C2*norm
            m_sb = pp.tile([P, NGRP], F32)
            b_sb = pp.tile([P, NGRP], F32)
            nc.vector.tensor_tensor(
                out=m_sb[:], in0=nl_sb[:], in1=nl_sb[:], op=mybir.AluOpType.mult
            )
            nc.vector.tensor_tensor(
                out=m_sb[:], in0=m_sb[:], in1=dg_sb[:], op=mybir.AluOpType.mult
            )
            nc.vector.tensor_scalar(
                out=m_sb[:],
                in0=m_sb[:],
                scalar1=-C2,
                scalar2=1.0,
                op0=mybir.AluOpType.mult,
                op1=mybir.AluOpType.add,
            )
            nc.vector.tensor_scalar(
                out=b_sb[:],
                in0=nl_sb[:],
                scalar1=C2,
                scalar2=None,
                op0=mybir.AluOpType.mult,
            )

            zst = None
            selt = None
            pt = None
            for s, (g, q, j) in enumerate(slot_list):
                gi, k = divmod(s, GS)
                if k == 0:
                    gs = min(GS, SLOTS - s)
                    zst = zp.tile([P, GS, D], FP8, tag="zst")
                    nc.sync.dma_start(
                        zst[:, 0:gs, :].rearrange("p a b -> p (a b)"),
                        zs_d.ap()[:, s * D : (s + gs) * D],
                    )
                    selt = spp.tile([P, GS, W], FP8, tag="selt")
                    nc.scalar.dma_start(
                        selt[:, 0:gs, :].rearrange("p a b -> p (a b)"),
                        sp_d.ap()[:, s * W : (s + gs) * W],
                    )
                b = bank_of[g]
                h = 0 if q < 2 else 1
                if s == first_slot[(b, 0)]:
                    ptL = ppool.tile([64, 512], F32, tag="ptL")
                    ptH = ppool.tile([64, 512], F32, tag="ptH")
                g0 = b * GPB
                cols = slice((g - g0) * D, (g - g0 + 1) * D)
                pt = ptL if h == 0 else ptH
                off = (q % 2) * W
                nc.tensor.matmul(
                    pt[off : off + W, cols],
                    selt[:, k, :],
                    zst[:, k, :],
                    start=(j == 0),
                    stop=(j == int(slots_gq[g, q]) - 1),
                    skip_group_check=True,
                )
                if s == last_any[b]:
                    # drain bank: nbr = psum * b  (C2*norm_dst), then
                    # out = zl*m + nbr for these groups, stream out
                    g1 = min(NGRP, g0 + GPB)
                    ng = g1 - g0
                    ccols = slice(g0 * D, g1 * D)
                    for half, ptx in ((0, ptL), (1, ptH)):
                        prow = slice(half * 64, half * 64 + 64)
                        p3 = ptx[:, 0 : ng * D].rearrange(
                            "p (a b) -> p a b", b=D
                        )
                        n3 = nbr_sb[prow, ccols].rearrange(
                            "p (a b) -> p a b", b=D
                        )
                        nc.vector.tensor_tensor(
                            out=n3,
                            in0=p3,
                            in1=b_sb[prow, g0:g1].to_broadcast([64, ng, D]),
                            op=mybir.AluOpType.mult,
                        )
                    zl3 = zl_sb[:, ccols].rearrange("p (a b) -> p a b", b=D)
                    o3 = out_sb[:, ccols].rearrange("p (a b) -> p a b", b=D)
                    nc.vector.tensor_tensor(
                        out=o3,
                        in0=zl3,
                        in1=m_sb[:, g0:g1].to_broadcast([P, ng, D]),
                        op=mybir.AluOpType.mult,
                    )
                    nc.vector.tensor_tensor(
                        out=out_sb[:, ccols],
                        in0=out_sb[:, ccols],
                        in1=nbr_sb[:, ccols],
                        op=mybir.AluOpType.add,
                    )
                    nc.scalar.dma_start(out_d.ap()[:, ccols], out_sb[:, ccols])

    return nc


def kernel(**inputs):
    global LAST_RESULTS
    z = np.asarray(inputs["z"], np.float32)
    edge_index = inputs["edge_index"]
    norm_factor = np.asarray(inputs["norm_factor"], np.float32)

    in_maps, meta = _preprocess(z, edge_index, norm_factor)

    nc = build_graph(meta)
    nc.compile()

    trace = os.environ.get("KERNEL_TRACE", "0") == "1"
    res = run_bass_kernel_spmd(
        nc, in_maps, core_ids=list(range(M)), trace=trace
    )
    LAST_RESULTS = res

    perm = meta["perm"]
    result = np.empty((N, D), np.float32)
    for c in range(M):
        o = np.asarray(res.results[c]["out"], np.float32)
        o = o.reshape(P, NGRP, D).transpose(1, 0, 2).reshape(NGRP * P, D)
        ids = np.arange(c * NPC, (c + 1) * NPC)
        result[ids] = o[perm[ids]]
    return result


# revision 29
# speedup vs baseline: 1.3577x; 1.2426x over previous
"""Trainium2 Bass kernel: Lap-regularizer gradient step (graph Laplacian).

out = z - COEFF * grad,  grad = (2/N) * norm ⊙ (deg·z_reg - A_sym·z_reg),
z_reg = norm ⊙ z, A_sym = symmetrized adjacency from edge_index.

Strategy (8 NeuronCores, SPMD, no collectives):
  - nodes sharded 12500/core; the 3.2M symmetrized directed edges are
    bucketed by dst core, grouped by (128-dst group, aligned 32-dst
    quarter), sorted, and packed into 128-edge chunks (partition-major).
  - per-edge streams from host (indexing/casts only, no arithmetic):
    z[src] rows (fp8, 48B), dst_rel (int16, 2B), norm[src] (bf16, 2B).
  - device builds the norm-scaled one-hot selectors with two DVE ops per
    stream tile (is_equal against a materialized iota, then mult), then
    segment-sums via bf16-sel x fp8-z matmuls accumulating in PSUM.
  - PSUM: one 512-f32 bank accumulates 10 dst groups (has_written
    overwrite-then-accumulate semantics); one DVE drain per bank with
    the C2*norm_dst scale fused; quarters issue round-robin so
    consecutive matmuls target different PE column groups.
"""

import os

import numpy as np
import ml_dtypes

import concourse.bass as bass
import concourse.mybir as mybir
import concourse.tile as tile
from concourse import bacc
from concourse._compat import get_trn_type
from concourse.bass_utils import run_bass_kernel_spmd

# Problem constants (hardcoded; kernel.py must be self-contained).
N = 100000
D = 48
COEFF = 0.1
C2 = COEFF * 2.0 / N

M = 8                      # cores
NPC = N // M               # nodes per core
P = 128
NGRP = (NPC + P - 1) // P  # 98 dst groups per core
W = 32                     # one-hot window = aligned 32-dst quarter
GS = 128                   # stream slots (of 128 edges) per DMA tile
GPB = 10                   # dst groups per PSUM bank (10*48=480 <= 512)

F32 = mybir.dt.float32
BF16 = mybir.dt.bfloat16
FP8 = mybir.dt.float8e4
I16 = mybir.dt.int16

LAST_RESULTS = None


def _slot_schedule(slots_gq):
    """Quarter-sequential slot order, shared by host packing and the
    device graph. Returns (slot_list, slot_of[g][q][j])."""
    slot_list = []
    slot_of = {}
    for g in range(NGRP):
        for q in range(4):
            for j in range(int(slots_gq[g, q])):
                slot_of[(g, q, j)] = len(slot_list)
                slot_list.append((g, q, j))
    return slot_list, slot_of


def _preprocess(z, edge_index, norm_factor):
    """Host-side sharding/packing. Returns per-core input maps + metadata."""
    ei = np.asarray(edge_index).astype(np.int64)
    row, col = ei[0], ei[1]
    src_all = np.concatenate([row, col])
    dst_all = np.concatenate([col, row])
    ne = src_all.shape[0]

    deg_all = np.bincount(dst_all, minlength=N)
    # pack each core's nodes into 392 windows of <=32 nodes, balancing the
    # per-window edge counts (LPT) so chunk counts are uniform across
    # (core, group, quarter) buckets -> minimal stream padding
    import heapq

    NW = NGRP * 4
    NOVER = 12                # big windows absorbing the highest-degree nodes
    ECAP = 8 * 128            # edge cap keeping regular windows at 8 chunks
    perm = np.empty(N, np.int64)   # node -> local slot within its core
    for c in range(M):
        ids = np.arange(c * NPC, (c + 1) * NPC)
        order_d = ids[np.argsort(-deg_all[ids], kind="stable")]
        counts = np.zeros(NW, np.int64)
        # top-degree nodes fill the overflow windows outright
        for i, v in enumerate(order_d[: NOVER * 32]):
            w = i // 32
            perm[v] = (w // 4) * 128 + (w % 4) * 32 + counts[w]
            counts[w] += 1
        # remaining nodes: LPT over capped windows (reject-on-overflow)
        heap = [(0, w) for w in range(NOVER, NW)]
        heapq.heapify(heap)
        for v in order_d[NOVER * 32 :]:
            dv = int(deg_all[v])
            stash = []
            while True:
                if not heap:
                    # infeasible tail: fall back to least-loaded stashed
                    heap = stash
                    heapq.heapify(heap)
                    stash = []
                    ssum, w = heapq.heappop(heap)
                    break
                ssum, w = heapq.heappop(heap)
                if counts[w] >= 32:
                    continue
                if ssum + dv > ECAP:
                    stash.append((ssum, w))
                    continue
                break
            for e in stash:
                heapq.heappush(heap, e)
            perm[v] = (w // 4) * 128 + (w % 4) * 32 + counts[w]
            counts[w] += 1
            if counts[w] < 32:
                heapq.heappush(heap, (ssum + dv, w))

    core = dst_all // NPC
    dloc = perm[dst_all]
    grp = dloc >> 7
    pdst = dloc & 127
    qrt = pdst >> 5

    # bucket = (core, group, quarter); chunks never straddle quarters so the
    # matmul PSUM base partition stays 32-aligned.
    NB = NGRP * 4
    key = (core * NGRP + grp) * 4 + qrt
    cnt = np.bincount(key, minlength=M * NB)
    bstart = np.zeros(M * NB + 1, np.int64)
    np.cumsum(cnt, out=bstart[1:])

    order = np.argsort(key, kind="stable")
    key_s = key[order]
    pdst_s = pdst[order]
    src_s = src_all[order]
    core_s = core[order]
    j_in = np.arange(ne, dtype=np.int64) - bstart[key_s]
    chunk_s = j_in >> 7
    p_s = (j_in & 127).astype(np.int64)

    # unified slot layout across cores: slots per (group, quarter), even
    # (DoubleRow consumes slot pairs) with a >=2 floor so every PSUM
    # element is written at least once (no stale PSUM reads at drains)
    K = ((cnt + 127) // 128).reshape(M, NB)
    slots_gq = np.maximum(K.max(axis=0), 1).reshape(NGRP, 4)
    slots_gq = (slots_gq + 1) // 2 * 2

    slot_list, slot_of = _slot_schedule(slots_gq)
    SLOTS = len(slot_list)
    SL = np.zeros((NGRP, 4, int(slots_gq.max())), np.int64)
    for (g, q, j), s in slot_of.items():
        SL[g, q, j] = s

    grp_s = key_s // 4 % NGRP
    qrt_s = key_s % 4
    slot_s = SL[grp_s, qrt_s, chunk_s]
    dl_rel = (pdst_s & 31).astype(np.int64)

    zf = np.asarray(z, np.float32)
    nf = np.asarray(norm_factor, np.float32).reshape(-1)
    zb = zf.astype(ml_dtypes.float8_e4m3)
    n8 = nf.astype(ml_dtypes.float8_e4m3)

    # per-edge source streams (host gather = indexing/casting only):
    # z rows (fp8) + norm[src]-scaled one-hot selector chunks (fp8)
    zs_arr = np.zeros((M, P, SLOTS, D), ml_dtypes.float8_e4m3)
    sp_arr = np.zeros((M, P, SLOTS * W), ml_dtypes.float8_e4m3)
    zs_arr[core_s, p_s, slot_s] = zb[src_s]
    sp_arr[core_s, p_s, slot_s * W + dl_rel] = n8[src_s]

    deg = deg_all.astype(np.float32)
    node_core = np.arange(N) // NPC

    def core_layout(x, width, dt):
        xp = np.zeros((M, NGRP * P, width), dt)
        xp[node_core, perm] = x.reshape(N, width).astype(dt)
        return (
            xp.reshape(M, NGRP, P, width)
            .transpose(0, 2, 1, 3)
            .reshape(M, P, NGRP * width)
        )

    zl_arr = core_layout(zf, D, np.float32)
    nl_arr = core_layout(nf.reshape(N, 1), 1, np.float32)
    dg_arr = core_layout(deg.reshape(N, 1), 1, np.float32)

    in_maps = []
    for c in range(M):
        in_maps.append(
            {
                "zs": np.ascontiguousarray(zs_arr[c]).reshape(P, SLOTS * D),
                "sp": np.ascontiguousarray(sp_arr[c]),
                "zl": np.ascontiguousarray(zl_arr[c]),
                "nl": np.ascontiguousarray(nl_arr[c]),
                "dg": np.ascontiguousarray(dg_arr[c]),
            }
        )

    meta = {"perm": perm, "SLOTS": SLOTS, "slots_gq": slots_gq}
    return in_maps, meta


def build_graph(meta):
    SLOTS = meta["SLOTS"]
    slots_gq = meta["slots_gq"]
    slot_list, _ = _slot_schedule(slots_gq)
    assert len(slot_list) == SLOTS

    # group -> psum superbank id; first/last pair ids per bank
    bank_of = [g // GPB for g in range(NGRP)]
    NPAIR = SLOTS // 2
    first_pair = {}
    last_pair = {}
    for t in range(NPAIR):
        g, q, j = slot_list[2 * t]
        b = bank_of[g]
        if b not in first_pair:
            first_pair[b] = t
        last_pair[b] = t

    nc = bacc.Bacc(
        get_trn_type() or "TRN2",
        target_bir_lowering=False,
        debug=False,
        num_devices=M,
    )

    zs_d = nc.dram_tensor("zs", [P, SLOTS * D], FP8, kind="ExternalInput")
    sp_d = nc.dram_tensor("sp", [P, SLOTS * W], FP8, kind="ExternalInput")
    zl_d = nc.dram_tensor("zl", [P, NGRP * D], F32, kind="ExternalInput")
    nl_d = nc.dram_tensor("nl", [P, NGRP], F32, kind="ExternalInput")
    dg_d = nc.dram_tensor("dg", [P, NGRP], F32, kind="ExternalInput")
    out_d = nc.dram_tensor("out", [P, NGRP * D], F32, kind="ExternalOutput")

    with tile.TileContext(nc) as tc:
        with tc.tile_pool(name="persist", bufs=1) as pp, tc.tile_pool(
            name="zstream", bufs=4
        ) as zp, tc.tile_pool(
            name="sstream", bufs=4
        ) as spp, tc.tile_pool(
            name="psum", bufs=2, space="PSUM"
        ) as ppool:
            zl_sb = pp.tile([P, NGRP * D], F32)
            nc.scalar.dma_start(zl_sb[:], zl_d.ap())
            nl_sb = pp.tile([P, NGRP], F32)
            nc.scalar.dma_start(nl_sb[:], nl_d.ap())
            dg_sb = pp.tile([P, NGRP], F32)
            nc.scalar.dma_start(dg_sb[:], dg_d.ap())
            out_sb = pp.tile([P, NGRP * D], F32)
            nbr_sb = pp.tile([P, NGRP * D], BF16)

            # m = 1 - C2*deg*norm^2 ; b = C2*norm
            m_sb = pp.tile([P, NGRP], F32)
            b_sb = pp.tile([P, NGRP], F32)
            nc.vector.tensor_tensor(
                out=m_sb[:], in0=nl_sb[:], in1=nl_sb[:], op=mybir.AluOpType.mult
            )
            nc.vector.tensor_tensor(
                out=m_sb[:], in0=m_sb[:], in1=dg_sb[:], op=mybir.AluOpType.mult
            )
            nc.vector.tensor_scalar(
                out=m_sb[:],
                in0=m_sb[:],
                scalar1=-C2,
                scalar2=1.0,
                op0=mybir.AluOpType.mult,
                op1=mybir.AluOpType.add,
            )
            nc.vector.tensor_scalar(
                out=b_sb[:],
                in0=nl_sb[:],
                scalar1=C2,
                scalar2=None,
                op0=mybir.AluOpType.mult,
            )

            GP = GS // 2          # pairs per stream tile
            zst = None
            selt = None
            pts = None
            for t in range(NPAIR):
                g, q, j = slot_list[2 * t]
                gi, kp = divmod(t, GP)
                if kp == 0:
                    s = 2 * t
                    gs = min(GS, SLOTS - s)
                    zst = zp.tile([P, GP, 2, D], FP8, tag="zst")
                    nc.sync.dma_start(
                        zst[:].rearrange("p a b c -> p (a b c)")[
                            :, 0 : gs * D
                        ],
                        zs_d.ap()[:, s * D : (s + gs) * D],
                    )
                    selt = spp.tile([P, GP, 2, W], FP8, tag="selt")
                    nc.scalar.dma_start(
                        selt[:].rearrange("p a b c -> p (a b c)")[
                            :, 0 : gs * W
                        ],
                        sp_d.ap()[:, s * W : (s + gs) * W],
                    )
                b = bank_of[g]
                if t == first_pair[b]:
                    pts = []
                    for qq in range(4):
                        ptq = ppool.tile(
                            [W, 512], F32, tag=f"pt{qq}", name=f"ptq{qq}"
                        )
                        pts.append(ptq)
                g0 = b * GPB
                cols = slice((g - g0) * D, (g - g0 + 1) * D)
                nc.tensor.matmul(
                    pts[q][:, cols],
                    selt[:, kp, :, :],
                    zst[:, kp, :, :],
                    start=(j == 0),
                    stop=(j == int(slots_gq[g, q]) - 2),
                    skip_group_check=True,
                    perf_mode=mybir.MatmulPerfMode.DoubleRow,
                )
                if t == last_pair[b]:
                    # drain bank: nbr = psum * b  (C2*norm_dst), then
                    # out = zl*m + nbr for these groups, stream out
                    g1 = min(NGRP, g0 + GPB)
                    ng = g1 - g0
                    ccols = slice(g0 * D, g1 * D)
                    for qq in range(4):
                        prow = slice(qq * W, qq * W + W)
                        p3 = pts[qq][:, 0 : ng * D].rearrange(
                            "p (a b) -> p a b", b=D
                        )
                        n3 = nbr_sb[prow, ccols].rearrange(
                            "p (a b) -> p a b", b=D
                        )
                        nc.vector.tensor_tensor(
                            out=n3,
                            in0=p3,
                            in1=b_sb[prow, g0:g1].to_broadcast([W, ng, D]),
                            op=mybir.AluOpType.mult,
                        )
                    zl3 = zl_sb[:, ccols].rearrange("p (a b) -> p a b", b=D)
                    o3 = out_sb[:, ccols].rearrange("p (a b) -> p a b", b=D)
                    nc.vector.tensor_tensor(
                        out=o3,
                        in0=zl3,
                        in1=m_sb[:, g0:g1].to_broadcast([P, ng, D]),
                        op=mybir.AluOpType.mult,
                    )
                    nc.vector.tensor_tensor(
                        out=out_sb[:, ccols],
                        in0=out_sb[:, ccols],
                        in1=nbr_sb[:, ccols],
                        op=mybir.AluOpType.add,
                    )
                    nc.scalar.dma_start(out_d.ap()[:, ccols], out_sb[:, ccols])

    return nc


def kernel(**inputs):
    global LAST_RESULTS
    z = np.asarray(inputs["z"], np.float32)
    edge_index = inputs["edge_index"]
    norm_factor = np.asarray(inputs["norm_factor"], np.float32)

    in_maps, meta = _preprocess(z, edge_index, norm_factor)

    nc = build_graph(meta)
    nc.compile()

    trace = os.environ.get("KERNEL_TRACE", "0") == "1"
    res = run_bass_kernel_spmd(
        nc, in_maps, core_ids=list(range(M)), trace=trace
    )
    LAST_RESULTS = res

    perm = meta["perm"]
    result = np.empty((N, D), np.float32)
    for c in range(M):
        o = np.asarray(res.results[c]["out"], np.float32)
        o = o.reshape(P, NGRP, D).transpose(1, 0, 2).reshape(NGRP * P, D)
        ids = np.arange(c * NPC, (c + 1) * NPC)
        result[ids] = o[perm[ids]]
    return result


# revision 30
# speedup vs baseline: 1.3790x; 1.0157x over previous
"""Trainium2 Bass kernel: Lap-regularizer gradient step (graph Laplacian).

out = z - COEFF * grad,  grad = (2/N) * norm ⊙ (deg·z_reg - A_sym·z_reg),
z_reg = norm ⊙ z, A_sym = symmetrized adjacency from edge_index.

Strategy (8 NeuronCores, SPMD, no collectives):
  - nodes sharded 12500/core; the 3.2M symmetrized directed edges are
    bucketed by dst core, grouped by (128-dst group, aligned 32-dst
    quarter), sorted, and packed into 128-edge chunks (partition-major).
  - per-edge streams from host (indexing/casts only, no arithmetic):
    z[src] rows (fp8, 48B), dst_rel (int16, 2B), norm[src] (bf16, 2B).
  - device builds the norm-scaled one-hot selectors with two DVE ops per
    stream tile (is_equal against a materialized iota, then mult), then
    segment-sums via bf16-sel x fp8-z matmuls accumulating in PSUM.
  - PSUM: one 512-f32 bank accumulates 10 dst groups (has_written
    overwrite-then-accumulate semantics); one DVE drain per bank with
    the C2*norm_dst scale fused; quarters issue round-robin so
    consecutive matmuls target different PE column groups.
"""

import os

import numpy as np
import ml_dtypes

import concourse.bass as bass
import concourse.mybir as mybir
import concourse.tile as tile
from concourse import bacc
from concourse._compat import get_trn_type
from concourse.bass_utils import run_bass_kernel_spmd

# Problem constants (hardcoded; kernel.py must be self-contained).
N = 100000
D = 48
COEFF = 0.1
C2 = COEFF * 2.0 / N

M = 8                      # cores
NPC = N // M               # nodes per core
P = 128
NGRP = (NPC + P - 1) // P  # 98 dst groups per core
W = 32                     # one-hot window = aligned 32-dst quarter
GS = 128                   # stream slots (of 128 edges) per DMA tile
GPB = 10                   # dst groups per PSUM bank (10*48=480 <= 512)

F32 = mybir.dt.float32
BF16 = mybir.dt.bfloat16
FP8 = mybir.dt.float8e4
I16 = mybir.dt.int16

LAST_RESULTS = None


def _slot_schedule(slots_gq):
    """Quarter-sequential slot order, shared by host packing and the
    device graph. Returns (slot_list, slot_of[g][q][j])."""
    slot_list = []
    slot_of = {}
    for g in range(NGRP):
        for q in range(4):
            for j in range(int(slots_gq[g, q])):
                slot_of[(g, q, j)] = len(slot_list)
                slot_list.append((g, q, j))
    return slot_list, slot_of


def _preprocess(z, edge_index, norm_factor):
    """Host-side sharding/packing. Returns per-core input maps + metadata."""
    ei = np.asarray(edge_index).astype(np.int64)
    row, col = ei[0], ei[1]
    src_all = np.concatenate([row, col])
    dst_all = np.concatenate([col, row])
    ne = src_all.shape[0]

    deg_all = np.bincount(dst_all, minlength=N)
    # pack each core's nodes into 392 windows of <=32 nodes, balancing the
    # per-window edge counts (LPT) so chunk counts are uniform across
    # (core, group, quarter) buckets -> minimal stream padding
    import heapq

    NW = NGRP * 4
    NOVER = 12                # big windows absorbing the highest-degree nodes
    ECAP = 8 * 128            # edge cap keeping regular windows at 8 chunks
    perm = np.empty(N, np.int64)   # node -> local slot within its core
    for c in range(M):
        ids = np.arange(c * NPC, (c + 1) * NPC)
        order_d = ids[np.argsort(-deg_all[ids], kind="stable")]
        counts = np.zeros(NW, np.int64)
        # top-degree nodes fill the overflow windows outright
        for i, v in enumerate(order_d[: NOVER * 32]):
            w = i // 32
            perm[v] = (w // 4) * 128 + (w % 4) * 32 + counts[w]
            counts[w] += 1
        # remaining nodes: LPT over capped windows (reject-on-overflow)
        heap = [(0, w) for w in range(NOVER, NW)]
        heapq.heapify(heap)
        for v in order_d[NOVER * 32 :]:
            dv = int(deg_all[v])
            stash = []
            while True:
                if not heap:
                    # infeasible tail: fall back to least-loaded stashed
                    heap = stash
                    heapq.heapify(heap)
                    stash = []
                    ssum, w = heapq.heappop(heap)
                    break
                ssum, w = heapq.heappop(heap)
                if counts[w] >= 32:
                    continue
                if ssum + dv > ECAP:
                    stash.append((ssum, w))
                    continue
                break
            for e in stash:
                heapq.heappush(heap, e)
            perm[v] = (w // 4) * 128 + (w % 4) * 32 + counts[w]
            counts[w] += 1
            if counts[w] < 32:
                heapq.heappush(heap, (ssum + dv, w))

    core = dst_all // NPC
    dloc = perm[dst_all]
    grp = dloc >> 7
    pdst = dloc & 127
    qrt = pdst >> 5

    # bucket = (core, group, quarter); chunks never straddle quarters so the
    # matmul PSUM base partition stays 32-aligned.
    NB = NGRP * 4
    key = (core * NGRP + grp) * 4 + qrt
    cnt = np.bincount(key, minlength=M * NB)
    bstart = np.zeros(M * NB + 1, np.int64)
    np.cumsum(cnt, out=bstart[1:])

    order = np.argsort(key, kind="stable")
    key_s = key[order]
    pdst_s = pdst[order]
    src_s = src_all[order]
    core_s = core[order]
    j_in = np.arange(ne, dtype=np.int64) - bstart[key_s]
    chunk_s = j_in >> 7
    p_s = (j_in & 127).astype(np.int64)

    # unified slot layout across cores: slots per (group, quarter), even
    # (DoubleRow consumes slot pairs) with a >=2 floor so every PSUM
    # element is written at least once (no stale PSUM reads at drains)
    K = ((cnt + 127) // 128).reshape(M, NB)
    slots_gq = np.maximum(K.max(axis=0), 1).reshape(NGRP, 4)
    slots_gq = (slots_gq + 1) // 2 * 2

    slot_list, slot_of = _slot_schedule(slots_gq)
    SLOTS = len(slot_list)
    SL = np.zeros((NGRP, 4, int(slots_gq.max())), np.int64)
    for (g, q, j), s in slot_of.items():
        SL[g, q, j] = s

    grp_s = key_s // 4 % NGRP
    qrt_s = key_s % 4
    slot_s = SL[grp_s, qrt_s, chunk_s]
    dl_rel = (pdst_s & 31).astype(np.int64)

    zf = np.asarray(z, np.float32)
    nf = np.asarray(norm_factor, np.float32).reshape(-1)
    zb = zf.astype(ml_dtypes.float8_e4m3)
    n8 = nf.astype(ml_dtypes.float8_e4m3)

    # per-edge source streams (host gather = indexing/casting only):
    # z rows (fp8) + norm[src]-scaled one-hot selector chunks (fp8)
    zs_arr = np.zeros((M, P, SLOTS, D), ml_dtypes.float8_e4m3)
    sp_arr = np.zeros((M, P, SLOTS * W), ml_dtypes.float8_e4m3)
    zs_arr[core_s, p_s, slot_s] = zb[src_s]
    sp_arr[core_s, p_s, slot_s * W + dl_rel] = n8[src_s]

    deg = deg_all.astype(np.float32)
    node_core = np.arange(N) // NPC

    def core_layout(x, width, dt):
        xp = np.zeros((M, NGRP * P, width), dt)
        xp[node_core, perm] = x.reshape(N, width).astype(dt)
        return (
            xp.reshape(M, NGRP, P, width)
            .transpose(0, 2, 1, 3)
            .reshape(M, P, NGRP * width)
        )

    zl_arr = core_layout(zf, D, np.float32)
    nl_arr = core_layout(nf.reshape(N, 1), 1, np.float32)
    dg_arr = core_layout(deg.reshape(N, 1), 1, np.float32)

    in_maps = []
    for c in range(M):
        in_maps.append(
            {
                "zs": np.ascontiguousarray(zs_arr[c]).reshape(P, SLOTS * D),
                "sp": np.ascontiguousarray(sp_arr[c]),
                "zl": np.ascontiguousarray(zl_arr[c]),
                "nl": np.ascontiguousarray(nl_arr[c]),
                "dg": np.ascontiguousarray(dg_arr[c]),
            }
        )

    meta = {"perm": perm, "SLOTS": SLOTS, "slots_gq": slots_gq}
    return in_maps, meta


def build_graph(meta):
    SLOTS = meta["SLOTS"]
    slots_gq = meta["slots_gq"]
    slot_list, _ = _slot_schedule(slots_gq)
    assert len(slot_list) == SLOTS

    # group -> psum superbank id; first/last pair ids per bank
    bank_of = [g // GPB for g in range(NGRP)]
    NPAIR = SLOTS // 2
    first_pair = {}
    last_pair = {}
    for t in range(NPAIR):
        g, q, j = slot_list[2 * t]
        b = bank_of[g]
        if b not in first_pair:
            first_pair[b] = t
        last_pair[b] = t

    nc = bacc.Bacc(
        get_trn_type() or "TRN2",
        target_bir_lowering=False,
        debug=False,
        num_devices=M,
    )

    zs_d = nc.dram_tensor("zs", [P, SLOTS * D], FP8, kind="ExternalInput")
    sp_d = nc.dram_tensor("sp", [P, SLOTS * W], FP8, kind="ExternalInput")
    zl_d = nc.dram_tensor("zl", [P, NGRP * D], F32, kind="ExternalInput")
    nl_d = nc.dram_tensor("nl", [P, NGRP], F32, kind="ExternalInput")
    dg_d = nc.dram_tensor("dg", [P, NGRP], F32, kind="ExternalInput")
    out_d = nc.dram_tensor("out", [P, NGRP * D], F32, kind="ExternalOutput")

    with tile.TileContext(nc) as tc:
        with tc.tile_pool(name="persist", bufs=1) as pp, tc.tile_pool(
            name="zstream", bufs=8
        ) as zp, tc.tile_pool(
            name="sstream", bufs=8
        ) as spp, tc.tile_pool(
            name="psum", bufs=2, space="PSUM"
        ) as ppool:
            zl_sb = pp.tile([P, NGRP * D], F32)
            nc.scalar.dma_start(zl_sb[:], zl_d.ap())
            nl_sb = pp.tile([P, NGRP], F32)
            nc.scalar.dma_start(nl_sb[:], nl_d.ap())
            dg_sb = pp.tile([P, NGRP], F32)
            nc.scalar.dma_start(dg_sb[:], dg_d.ap())
            out_sb = pp.tile([P, NGRP * D], F32)
            nbr_sb = pp.tile([P, NGRP * D], BF16)

            # m = 1 - C2*deg*norm^2 ; b = C2*norm
            m_sb = pp.tile([P, NGRP], F32)
            b_sb = pp.tile([P, NGRP], F32)
            nc.vector.tensor_tensor(
                out=m_sb[:], in0=nl_sb[:], in1=nl_sb[:], op=mybir.AluOpType.mult
            )
            nc.vector.tensor_tensor(
                out=m_sb[:], in0=m_sb[:], in1=dg_sb[:], op=mybir.AluOpType.mult
            )
            nc.vector.tensor_scalar(
                out=m_sb[:],
                in0=m_sb[:],
                scalar1=-C2,
                scalar2=1.0,
                op0=mybir.AluOpType.mult,
                op1=mybir.AluOpType.add,
            )
            nc.vector.tensor_scalar(
                out=b_sb[:],
                in0=nl_sb[:],
                scalar1=C2,
                scalar2=None,
                op0=mybir.AluOpType.mult,
            )

            GP = GS // 2          # pairs per stream tile
            zst = None
            selt = None
            pts = None
            for t in range(NPAIR):
                g, q, j = slot_list[2 * t]
                gi, kp = divmod(t, GP)
                if kp == 0:
                    s = 2 * t
                    gs = min(GS, SLOTS - s)
                    zst = zp.tile([P, GP, 2, D], FP8, tag="zst")
                    nc.sync.dma_start(
                        zst[:].rearrange("p a b c -> p (a b c)")[
                            :, 0 : gs * D
                        ],
                        zs_d.ap()[:, s * D : (s + gs) * D],
                    )
                    selt = spp.tile([P, GP, 2, W], FP8, tag="selt")
                    nc.scalar.dma_start(
                        selt[:].rearrange("p a b c -> p (a b c)")[
                            :, 0 : gs * W
                        ],
                        sp_d.ap()[:, s * W : (s + gs) * W],
                    )
                b = bank_of[g]
                if t == first_pair[b]:
                    pts = []
                    for qq in range(4):
                        ptq = ppool.tile(
                            [W, 512], F32, tag=f"pt{qq}", name=f"ptq{qq}"
                        )
                        pts.append(ptq)
                g0 = b * GPB
                cols = slice((g - g0) * D, (g - g0 + 1) * D)
                nc.tensor.matmul(
                    pts[q][:, cols],
                    selt[:, kp, :, :],
                    zst[:, kp, :, :],
                    start=(j == 0),
                    stop=(j == int(slots_gq[g, q]) - 2),
                    skip_group_check=True,
                    perf_mode=mybir.MatmulPerfMode.DoubleRow,
                )
                if t == last_pair[b]:
                    # drain bank: nbr = psum * b  (C2*norm_dst), then
                    # out = zl*m + nbr for these groups, stream out
                    g1 = min(NGRP, g0 + GPB)
                    ng = g1 - g0
                    ccols = slice(g0 * D, g1 * D)
                    for qq in range(4):
                        prow = slice(qq * W, qq * W + W)
                        p3 = pts[qq][:, 0 : ng * D].rearrange(
                            "p (a b) -> p a b", b=D
                        )
                        n3 = nbr_sb[prow, ccols].rearrange(
                            "p (a b) -> p a b", b=D
                        )
                        nc.vector.tensor_tensor(
                            out=n3,
                            in0=p3,
                            in1=b_sb[prow, g0:g1].to_broadcast([W, ng, D]),
                            op=mybir.AluOpType.mult,
                        )
                    zl3 = zl_sb[:, ccols].rearrange("p (a b) -> p a b", b=D)
                    o3 = out_sb[:, ccols].rearrange("p (a b) -> p a b", b=D)
                    nc.vector.tensor_tensor(
                        out=o3,
                        in0=zl3,
                        in1=m_sb[:, g0:g1].to_broadcast([P, ng, D]),
                        op=mybir.AluOpType.mult,
                    )
                    nc.vector.tensor_tensor(
                        out=out_sb[:, ccols],
                        in0=out_sb[:, ccols],
                        in1=nbr_sb[:, ccols],
                        op=mybir.AluOpType.add,
                    )
                    nc.scalar.dma_start(out_d.ap()[:, ccols], out_sb[:, ccols])

    return nc


def kernel(**inputs):
    global LAST_RESULTS
    z = np.asarray(inputs["z"], np.float32)
    edge_index = inputs["edge_index"]
    norm_factor = np.asarray(inputs["norm_factor"], np.float32)

    in_maps, meta = _preprocess(z, edge_index, norm_factor)

    nc = build_graph(meta)
    nc.compile()

    trace = os.environ.get("KERNEL_TRACE", "0") == "1"
    res = run_bass_kernel_spmd(
        nc, in_maps, core_ids=list(range(M)), trace=trace
    )
    LAST_RESULTS = res

    perm = meta["perm"]
    result = np.empty((N, D), np.float32)
    for c in range(M):
        o = np.asarray(res.results[c]["out"], np.float32)
        o = o.reshape(P, NGRP, D).transpose(1, 0, 2).reshape(NGRP * P, D)
        ids = np.arange(c * NPC, (c + 1) * NPC)
        result[ids] = o[perm[ids]]
    return result
